# revision 16
# baseline (speedup 1.0000x reference)
"""CAAM kernel for Trainium2: builder + host-side prep (bf16 matmul pipeline).

Per-core: one batch element. Key layouts:
  x resident as 4 SBUF tiles [128, 8192] bf16, BIN-BLOCKED on host
  (free index = n*1024 + ph*32 + pw), loaded with 4 contiguous DMAs.
  camE [32, 8192] bf16 (rows 19:32 zeroed): cam -> exp in place.
  Phase B uses DMA-transpose (XBAR, 16-bit) to produce pixel-partitioned
  ET [128, 8, 32] and xT [128, 8, 512] tiles — no PE transposes/copies.
  attnT is cached in SBUF (atile[n], bf16) through phase F, which therefore
  only redoes the out-projection matmul after BN stats are known.
  BN batch stats (sum, sumsq per channel) are allreduced across cores.
"""

import numpy as np
import ml_dtypes
import concourse.bass as bass
import concourse.mybir as mybir

F32 = mybir.dt.float32
BF16 = mybir.dt.bfloat16
NPBF16 = ml_dtypes.bfloat16
AX = mybir.AxisListType
OP = mybir.AluOpType
ACT = mybir.ActivationFunctionType

B, C, H, W = 8, 512, 64, 128
K, BH, BW = 19, 2, 4
NB = BH * BW          # 8
CI = C // 2           # 256
HWp = H * W           # 8192
RH, RW = H // BH, W // BW   # 32, 32
P = RH * RW           # 1024
CC = C // 128         # 4
IC = CI // 128        # 2
KN = K * NB           # 152
EPS = 1e-5

# -------- wpackE column map (fp32 consts) --------
E_IDN = 0        # 128 cols            identity (phase C transposes)
E_W1NK0 = 128    # 152 cols, rows 0:128  conv1 lhsT chunk0
E_W1NK1 = 280    # 152 cols, rows 0:24   conv1 lhsT chunk1
E_FNK0 = 432     # 19 cols, rows 0:128   fuse lhsT chunk0
E_FNK1 = 451     # 19 cols, rows 0:24    fuse lhsT chunk1
E_GANK = 470     # 2 cols: gcn_a-1 per stack row (chunk0, chunk1)
E_CAMB = 472     # 1 col, rows 0:19
E_FB = 473       # 1 col, rows 0:19      fuse_b
E_RAM1 = 474     # 1 col, rows 0:19      relu_a - 1
E_KB = 475       # 2 cols                k_b chunks
E_VB = 477       # 256 cols, row 0       v_b
E_ONE119 = 733   # 19 cols, row 0        ones
NEf = 752

# -------- wpackB column map (bf16 consts) --------
B_WCAM = 0       # 76 cols (4 chunks x 19)  conv_cam_w^T
B_ONEK = 76      # 1 col, rows 0:19  ones
B_ONE119 = 77    # 19 cols, row 0    ones
NBf = 96

# -------- wpackL column map (late fp32 consts, [128, 35]) --------
L_QB = 0         # 2 cols
L_GAMMA = 2      # 4
L_BETA = 6       # 4
L_OAM1 = 10      # 4  out_a - 1
L_EPS = 14       # 1
NL = 35

# -------- dsmallA ([128, 40]): phase A stats --------
A_CSUM = 0       # 8 cols, rows 0:19
A_ESUM = 8
A_CLS = 16
A_REC = 24
A_SCALE = 32     # 8 cols: cls * rec
NA = 40

# -------- dsmallDE ([128, 646]) --------
D_RS = 0         # 16: attnT row sums (ic, bin)
D_SQ = 16        # 64: y^2 sums (cc, bin, nh)
D_ST = 80        # 8: packed allreduce input (sum, sumsq per cc)
D_SBN = 88       # 8: allreduce output
D_SCOL = 96      # 4
D_BCOL = 100     # 4
D_RSUM = 104     # 2
D_MOM = 106      # 8
D_VAR = 114      # 4
D_MUSQ = 118     # 4
D_SD = 122       # 4
D_RSTD = 126     # 4
D_NSC = 130      # 4
ND = 646

# -------- scrC column map (phase-C scratch inside bigE slot) --------
S_VA = 0         # 512   prelu'd t, chunk0
S_VB = 512       # 512, rows 0:24  chunk1
S_UG = 1024      # 512
S_MG = 1536      # 512
S_TT = 2048      # 608 = 4 x 152
S_L2A = 2656     # 512
S_L2B = 3168     # 512, rows 0:24
S_GL = 3680      # 512, rows 0:19  glob (prelu'd)
S_UG2 = 4192     # 512
S_MG2 = 4704     # 512
S_GT = 5216      # 76 = 4 x 19
S_L2T = 5292     # 608
NS = 5900

# attw pack (bf16): keyT 0:304, val 304:560
AT_KEYT = 0
AT_VAL = 304
NAT = 560


def host_prep(wts: dict) -> dict:
    w1 = np.asarray(wts["gcn_w1"], np.float32)
    ga = np.asarray(wts["gcn_a"], np.float32)
    fw = np.asarray(wts["fuse_w"], np.float32).reshape(-1)
    fb = float(np.asarray(wts["fuse_b"], np.float32).reshape(-1)[0])
    ra = float(np.asarray(wts["relu_a"], np.float32).reshape(-1)[0])

    wE = np.zeros((128, NEf), np.float32)
    # conv1 lhsT: W[(m*19+kp), (n*19+k)] = w1[n, m] * (kp == k)
    W1NK = np.zeros((KN, KN), np.float32)
    FNK = np.zeros((KN, K), np.float32)
    ga_nk = np.zeros(KN, np.float32)
    for n in range(NB):
        for k in range(K):
            for m in range(NB):
                W1NK[m*K + k, n*K + k] = w1[n, m]
            FNK[n*K + k, k] = fw[n]
            ga_nk[n*K + k] = ga[n] - 1.0
    wE[:, E_IDN:E_IDN + 128] = np.eye(128, dtype=np.float32)
    wE[:, E_W1NK0:E_W1NK0 + KN] = W1NK[0:128]
    wE[0:24, E_W1NK1:E_W1NK1 + KN] = W1NK[128:KN]
    wE[:, E_FNK0:E_FNK0 + K] = FNK[0:128]
    wE[0:24, E_FNK1:E_FNK1 + K] = FNK[128:KN]
    wE[:, E_GANK] = ga_nk[0:128]
    wE[0:24, E_GANK + 1] = ga_nk[128:KN]
    wE[0:K, E_CAMB] = np.asarray(wts["conv_cam_b"], np.float32)
    wE[0:K, E_FB] = fb
    wE[0:K, E_RAM1] = ra - 1.0
    wE[:, E_KB:E_KB + 2] = np.asarray(wts["k_b"], np.float32).reshape(IC, 128).T
    wE[0, E_VB:E_VB + CI] = np.asarray(wts["v_b"], np.float32)
    wE[0, E_ONE119:E_ONE119 + K] = 1.0

    wB = np.zeros((128, NBf), np.float32)
    wcamT = np.asarray(wts["conv_cam_w"], np.float32).T    # [512, 19]
    for cc in range(CC):
        wB[:, B_WCAM + cc*K:B_WCAM + (cc+1)*K] = wcamT[cc*128:(cc+1)*128]
    wB[0:K, B_ONEK] = 1.0
    wB[0, B_ONE119:B_ONE119 + K] = 1.0

    wL = np.zeros((128, NL), np.float32)
    wL[:, L_QB:L_QB + 2] = np.asarray(wts["q_b"], np.float32).reshape(IC, 128).T
    wL[:, L_GAMMA:L_GAMMA + 4] = np.asarray(wts["bn_gamma"], np.float32).reshape(CC, 128).T
    wL[:, L_BETA:L_BETA + 4] = np.asarray(wts["bn_beta"], np.float32).reshape(CC, 128).T
    wL[:, L_OAM1:L_OAM1 + 4] = (np.asarray(wts["out_a"], np.float32) - 1.0).reshape(CC, 128).T
    wL[:, L_EPS] = EPS

    return {
        "wpackE": wE,
        "wpackB": wB.astype(NPBF16),
        "wpackL": wL,
        "w2T": np.ascontiguousarray(np.asarray(wts["gcn_w2"], np.float32).T),
        "kwT": np.ascontiguousarray(np.asarray(wts["k_w"], np.float32).T),
        "vwT": np.ascontiguousarray(np.asarray(wts["v_w"], np.float32).T),
        "qwT": np.ascontiguousarray(np.asarray(wts["q_w"], np.float32).T).astype(NPBF16),
        "outwT": np.ascontiguousarray(np.asarray(wts["out_w"], np.float32).T).astype(NPBF16),
    }


WEIGHT_SPECS = [
    ("wpackE", [128, NEf], F32), ("wpackB", [128, NBf], BF16),
    ("wpackL", [128, NL], F32),
    ("w2T", [C, C], F32), ("kwT", [C, CI], F32), ("vwT", [C, CI], F32),
    ("qwT", [C, CI], BF16), ("outwT", [CI, C], BF16),
]


def _load_chunked(nc, pool, ap, r, cdim, name, dt=F32):
    """DRAM [r, cdim] (r = n*128) -> SBUF [128, n*cdim], column-grouped."""
    nchunk = r // 128
    t = pool.tile([128, nchunk * cdim], dt, name=name)
    src = ap.rearrange("(n p) c -> p n c", p=128)
    nc.sync.dma_start(t[:].rearrange("p (n c) -> p n c", n=nchunk), src)
    return t


def build_caam(tc, outs, ins, n_cores, use_collective=True, stop_after=None):
    nc = tc.nc
    x_d = ins["x"]          # [C, HWp] bf16, bin-blocked on host
    y_d = outs["y"]
    Ntot = float(n_cores * HWp)
    yv = y_d.rearrange("c h w -> c (h w)")

    # ---------------- pool stack (LIFO) ----------------
    wpoolL = tc.alloc_tile_pool(name="wtsL", bufs=1)
    dpool = tc.alloc_tile_pool(name="phD", bufs=1)
    attw = tc.alloc_tile_pool(name="attw", bufs=1)
    xpool = tc.alloc_tile_pool(name="x_res", bufs=1)
    gpool = tc.alloc_tile_pool(name="gcn", bufs=1)
    wpoolE = tc.alloc_tile_pool(name="wtsE", bufs=1)

    live = [wpoolL, dpool, attw, xpool, gpool, wpoolE]

    wE = wpoolE.tile([128, NEf], F32, name="wpackE")
    nc.sync.dma_start(wE[:], ins["wpackE"])
    wB = wpoolL.tile([128, NBf], BF16, name="wpackB")
    nc.sync.dma_start(wB[:], ins["wpackB"])
    kwT = _load_chunked(nc, wpoolE, ins["kwT"], C, CI, "kwT")
    vwT = _load_chunked(nc, wpoolE, ins["vwT"], C, CI, "vwT")
    wL = wpoolL.tile([128, NL], F32, name="wpackL")
    nc.sync.dma_start(wL[:], ins["wpackL"])
    qwT = _load_chunked(nc, wpoolL, ins["qwT"], C, CI, "qwT", dt=BF16)
    outwT = _load_chunked(nc, wpoolL, ins["outwT"], CI, C, "outwT", dt=BF16)

    idn = wE[:, E_IDN:E_IDN + 128]

    dsA = dpool.tile([128, NA], F32, name="dsmallA")
    dsD = dpool.tile([128, ND], F32, name="dsmallDE")
    atile = [dpool.tile([128, IC * P], BF16, name=f"attn{n}") for n in range(NB)]

    def _finish_early():
        nc.sync.dma_start(yv[0:128, 0:512], dsD[:, 0:512])
        for p in reversed(live):
            p.release()

    # ---------------- phase A ----------------
    x_sb = []
    for cc in range(CC):
        t = xpool.tile([128, HWp], BF16, name=f"x_{cc}")
        nc.sync.dma_start(t[:], x_d[cc * 128:(cc + 1) * 128, :])
        x_sb.append(t)

    if stop_after == "load":
        _finish_early()
        return

    camE = dpool.tile([32, HWp], BF16, tag="bigE", name="camE")
    nc.vector.memset(camE[:, :], 0.0)
    with tc.tile_pool(name="phA_ps", bufs=1, space="PSUM") as aps:
        for n in range(NB):
            for nh in range(2):
                c0 = n * P + nh * 512
                cp = aps.tile([K, 512], F32, tag="camps", bufs=2)
                for cc in range(CC):
                    nc.tensor.matmul(cp[:], wB[:, B_WCAM + cc*K:B_WCAM + (cc+1)*K],
                                     x_sb[cc][:, c0:c0 + 512],
                                     start=(cc == 0), stop=(cc == CC - 1))
                nc.scalar.activation(camE[0:K, c0:c0 + 512], cp[:],
                                     ACT.Identity, bias=wE[0:K, E_CAMB:E_CAMB + 1])
            sl = camE[0:K, n * P:(n + 1) * P]
            nc.vector.tensor_reduce(dsA[0:K, A_CSUM + n:A_CSUM + n + 1], sl, axis=AX.X, op=OP.add)
            nc.scalar.activation(sl, sl, ACT.Exp)
            nc.vector.tensor_reduce(dsA[0:K, A_ESUM + n:A_ESUM + n + 1], sl, axis=AX.X, op=OP.add)
    nc.scalar.activation(dsA[0:K, A_CLS:A_CLS + NB], dsA[0:K, A_CSUM:A_CSUM + NB],
                         ACT.Sigmoid, scale=1.0 / P)
    nc.vector.reciprocal(dsA[0:K, A_REC:A_REC + NB], dsA[0:K, A_ESUM:A_ESUM + NB])
    nc.vector.tensor_mul(dsA[0:K, A_SCALE:A_SCALE + NB],
                         dsA[0:K, A_CLS:A_CLS + NB], dsA[0:K, A_REC:A_REC + NB])

    if stop_after == "A":
        _finish_early()
        return

    # ---------------- phase B: per-bin local ----------------
    # stack rows: p = n*19 + k; chunk0 rows 0:128 cols 0:512, chunk1 rows 0:24 cols 512:1024
    stack = gpool.tile([128, 2 * C], F32, name="stack")
    stackA = stack[:, 0:C]
    stackB = stack[0:24, C:2 * C]
    with tc.tile_pool(name="phB_sb", bufs=1) as bsb, \
         tc.tile_pool(name="phB_ps", bufs=1, space="PSUM") as bps:
        for n in range(NB):
            # DMA-transpose (XBAR): ET[p, pc, k] = camE[k, pc*128+p]; xT[p, pc, c] = x[c, pc*128+p]
            ET = bsb.tile([128, NB, 32], BF16, tag="ET", bufs=2)
            nc.sync.dma_start(ET[:], camE[:, n * P:(n + 1) * P], transpose=True)
            xT = bsb.tile([128, NB, C], BF16, tag="xT", bufs=2)
            for cc in range(CC):
                nc.sync.dma_start(xT[:, :, cc * 128:(cc + 1) * 128],
                                  x_sb[cc][:, n * P:(n + 1) * P], transpose=True)
            locp = bps.tile([K, C], F32, tag="locp", bufs=2)
            for pc in range(8):
                nc.tensor.matmul(locp[:], ET[:, pc, 0:K], xT[:, pc, :],
                                 start=(pc == 0), stop=(pc == 7))
            locS = bsb.tile([K, C], F32, tag="locS", bufs=2)
            nc.vector.tensor_single_scalar(locS[:], locp[:],
                                           dsA[0:K, A_SCALE + n:A_SCALE + n + 1], OP.mult)
            # stack rows n*19 .. n*19+19 (may straddle the chunk boundary at p=128)
            p0 = n * K
            p1 = p0 + K
            if p1 <= 128:
                nc.sync.dma_start(stackA[p0:p1, :], locS[:, :])
            elif p0 >= 128:
                nc.sync.dma_start(stackB[p0 - 128:p1 - 128, :], locS[:, :])
            else:
                nc.sync.dma_start(stackA[p0:128, :], locS[0:128 - p0, :])
                nc.sync.dma_start(stackB[0:p1 - 128, :], locS[128 - p0:K, :])

    if stop_after == "B":
        _finish_early()
        return

    # ---------------- phase C: GCN + fuse + key/val (fp32) ----------------
    atp = attw.tile([128, NAT], BF16, name="attpack")
    keyT = atp[:, AT_KEYT:AT_KEYT + IC * KN]
    val = atp[0:K, AT_VAL:AT_VAL + CI]
    scrC = dpool.tile([128, NS], F32, tag="bigE", name="scrC")
    vA = scrC[:, S_VA:S_VA + C]
    vB = scrC[0:24, S_VB:S_VB + C]
    with tc.tile_pool(name="phC_sb", bufs=1) as csb, \
         tc.tile_pool(name="phC_ps", bufs=1, space="PSUM") as cps:
        w2T = _load_chunked(nc, csb, ins["w2T"], C, C, "w2T")
        # conv1: t = W1NK.T @ stack  (contraction over 152 stack rows, 2 chunks)
        tpA = cps.tile([128, C], F32, tag="big")
        nc.tensor.matmul(tpA[:], wE[:, E_W1NK0:E_W1NK0 + 128], stackA, start=True, stop=False)
        nc.tensor.matmul(tpA[:], wE[0:24, E_W1NK1:E_W1NK1 + 128], stackB, start=False, stop=True)
        tpB = cps.tile([24, C], F32, tag="smallB")
        nc.tensor.matmul(tpB[:], wE[:, E_W1NK0 + 128:E_W1NK0 + KN], stackA, start=True, stop=False)
        nc.tensor.matmul(tpB[:], wE[0:24, E_W1NK1 + 128:E_W1NK1 + KN], stackB, start=False, stop=True)
        # prelu(t + stack) with per-row alpha = gcn_a[n] (E_GANK cols)
        for (tp, st, vv, gchunk, rows) in ((tpA, stackA, vA, 0, 128),
                                           (tpB, stackB, vB, 1, 24)):
            u_ = scrC[0:rows, S_UG:S_UG + C]
            nc.vector.tensor_add(u_, tp[:], st)
            m_ = scrC[0:rows, S_MG:S_MG + C]
            nc.vector.tensor_scalar_min(m_, u_, 0.0)
            nc.vector.scalar_tensor_tensor(vv, m_, wE[0:rows, E_GANK + gchunk:E_GANK + gchunk + 1],
                                           u_, OP.mult, OP.add)
        # transpose t -> tT [c, (n,k)]
        for cc in range(CC):
            tt = scrC[:, S_TT + cc * KN:S_TT + (cc + 1) * KN]
            pA = cps.tile([128, 128], F32, tag="trA")
            nc.tensor.transpose(pA[:], vA[:, cc * 128:(cc + 1) * 128], idn)
            nc.scalar.copy(tt[:, 0:128], pA[:])
            pB = cps.tile([128, 24], F32, tag="trB")
            nc.tensor.transpose(pB[:], vB[:, cc * 128:(cc + 1) * 128], idn[0:24, 0:24])
            nc.scalar.copy(tt[:, 128:152], pB[:])
        # w2: local2 = t @ w2T (stack layout out)
        l2A = scrC[:, S_L2A:S_L2A + C]
        l2B = scrC[0:24, S_L2B:S_L2B + C]
        pl2A = cps.tile([128, C], F32, tag="big")
        for cc in range(CC):
            nc.tensor.matmul(pl2A[:], scrC[:, S_TT + cc * KN:S_TT + cc * KN + 128],
                             w2T[:, cc * C:(cc + 1) * C], start=(cc == 0), stop=(cc == CC - 1))
        nc.scalar.copy(l2A, pl2A[:])
        pl2B = cps.tile([24, C], F32, tag="smallB")
        for cc in range(CC):
            nc.tensor.matmul(pl2B[:], scrC[:, S_TT + cc * KN + 128:S_TT + cc * KN + 152],
                             w2T[:, cc * C:(cc + 1) * C], start=(cc == 0), stop=(cc == CC - 1))
        nc.scalar.copy(l2B, pl2B[:])
        # fuse -> glob [19, 512] (one psum tile), then prelu
        gp = cps.tile([K, C], F32, tag="gAB")
        nc.tensor.matmul(gp[:], wE[:, E_FNK0:E_FNK0 + K], l2A, start=True, stop=False)
        nc.tensor.matmul(gp[:], wE[0:24, E_FNK1:E_FNK1 + K], l2B, start=False, stop=True)
        glob = scrC[0:K, S_GL:S_GL + C]
        u_ = scrC[0:K, S_UG2:S_UG2 + C]
        nc.vector.tensor_scalar_add(u_, gp[:], wE[0:K, E_FB:E_FB + 1])
        m_ = scrC[0:K, S_MG2:S_MG2 + C]
        nc.vector.tensor_scalar_min(m_, u_, 0.0)
        nc.vector.scalar_tensor_tensor(glob, m_, wE[0:K, E_RAM1:E_RAM1 + 1], u_, OP.mult, OP.add)
        # globT + val (+ v_b via ones-row matmul); val cast to bf16
        valp = cps.tile([K, CI], F32, tag="valp")
        for cc in range(CC):
            gt = scrC[:, S_GT + cc * K:S_GT + (cc + 1) * K]
            pA = cps.tile([128, K], F32, tag="trB")
            nc.tensor.transpose(pA[:], glob[:, cc * 128:(cc + 1) * 128], idn[0:K, 0:K])
            nc.scalar.copy(gt[:, :], pA[:])
            nc.tensor.matmul(valp[:], gt[:], vwT[:, cc * CI:(cc + 1) * CI],
                             start=(cc == 0), stop=False)
        nc.tensor.matmul(valp[:], wE[0:1, E_ONE119:E_ONE119 + K], wE[0:1, E_VB:E_VB + CI],
                         start=False, stop=True)
        nc.scalar.copy(val, valp[:])
        # local2T + keyT (+ k_b per-partition bias); keyT cast to bf16
        for cc in range(CC):
            lt = scrC[:, S_L2T + cc * KN:S_L2T + (cc + 1) * KN]
            pA = cps.tile([128, 128], F32, tag="trA")
            nc.tensor.transpose(pA[:], l2A[:, cc * 128:(cc + 1) * 128], idn)
            nc.scalar.copy(lt[:, 0:128], pA[:])
            pB = cps.tile([128, 24], F32, tag="trB")
            nc.tensor.transpose(pB[:], l2B[:, cc * 128:(cc + 1) * 128], idn[0:24, 0:24])
            nc.scalar.copy(lt[:, 128:152], pB[:])
        for ic in range(IC):
            kp = cps.tile([128, KN], F32, tag="keyp")
            for cc in range(CC):
                nc.tensor.matmul(kp[:], kwT[:, cc * CI + ic * 128: cc * CI + (ic + 1) * 128],
                                 scrC[:, S_L2T + cc * KN:S_L2T + (cc + 1) * KN],
                                 start=(cc == 0), stop=(cc == CC - 1))
            nc.scalar.activation(keyT[:, ic * KN:(ic + 1) * KN], kp[:], ACT.Identity,
                                 bias=wE[:, E_KB + ic:E_KB + ic + 1])
    wpoolE.release()
    gpool.release()
    live.remove(wpoolE)
    live.remove(gpool)

    if stop_after == "C":
        _finish_early()
        return

    # ---------------- phase D+E: attention + y stats ----------------
    with tc.tile_pool(name="phD_sb", bufs=1) as dsb, \
         tc.tile_pool(name="phD_ps", bufs=1, space="PSUM") as dps:
        for n in range(NB):
            qT = dsb.tile([128, IC * P], BF16, tag="qT", bufs=2)
            for ic in range(IC):
                for nh in range(2):
                    qp = dps.tile([128, 512], F32, tag="qp")
                    for cc in range(CC):
                        xsl = x_sb[cc][:, n * P + nh * 512: n * P + (nh + 1) * 512]
                        nc.tensor.matmul(qp[:], qwT[:, cc * CI + ic * 128: cc * CI + (ic + 1) * 128],
                                         xsl, start=(cc == 0), stop=(cc == CC - 1))
                    nc.scalar.activation(qT[:, ic * P + nh * 512: ic * P + (nh + 1) * 512], qp[:],
                                         ACT.Identity, bias=wL[:, L_QB + ic:L_QB + ic + 1])
            ebin = dsb.tile([K, P], BF16, tag="ebin", bufs=2)
            for nh in range(2):
                afp = dps.tile([K, 512], F32, tag="afp")
                for ic in range(IC):
                    ksel = keyT[:, ic * KN + n * K: ic * KN + (n + 1) * K]
                    nc.tensor.matmul(afp[:], ksel, qT[:, ic * P + nh * 512: ic * P + (nh + 1) * 512],
                                     start=(ic == 0), stop=(ic == IC - 1))
                nc.scalar.activation(ebin[:, nh * 512:(nh + 1) * 512], afp[:], ACT.Exp)
                sp = dps.tile([1, 512], F32, tag="nrm", bufs=2)
                nc.tensor.matmul(sp[:], wB[0:K, B_ONEK:B_ONEK + 1],
                                 ebin[:, nh * 512:(nh + 1) * 512], start=True, stop=True)
                rrow = dsb.tile([1, 512], BF16, tag="rrow", bufs=2)
                with nc.allow_low_precision(reason="softmax denom reciprocal; 2e-2 tol"):
                    nc.vector.reciprocal(rrow[:], sp[:])
                rbp = dps.tile([K, 512], F32, tag="nrm", bufs=2)
                nc.tensor.matmul(rbp[:], wB[0:1, B_ONE119:B_ONE119 + K], rrow[:], start=True, stop=True)
                nc.vector.tensor_mul(ebin[:, nh * 512:(nh + 1) * 512],
                                     ebin[:, nh * 512:(nh + 1) * 512], rbp[:])
            for ic in range(IC):
                aop = dps.tile([128, P], F32, tag="aop")
                for nh in range(2):
                    nc.tensor.matmul(aop[:, nh * 512:(nh + 1) * 512], val[:, ic * 128:(ic + 1) * 128],
                                     ebin[:, nh * 512:(nh + 1) * 512], start=True, stop=True)
                nc.scalar.activation(atile[n][:, ic * P:(ic + 1) * P], aop[:], ACT.Copy,
                                     accum_out=dsD[:, D_RS + ic * NB + n: D_RS + ic * NB + n + 1])
            for cc in range(CC):
                for nh in range(2):
                    yp = dps.tile([128, 512], F32, tag="yp", bufs=2)
                    for ic in range(IC):
                        nc.tensor.matmul(yp[:], outwT[:, ic * C + cc * 128: ic * C + (cc + 1) * 128],
                                         atile[n][:, ic * P + nh * 512: ic * P + (nh + 1) * 512],
                                         start=(ic == 0), stop=(ic == IC - 1))
                    ysq = dsb.tile([128, 512], BF16, tag="ysq", bufs=2)
                    col = D_SQ + cc * 2 * NB + n * 2 + nh
                    nc.scalar.activation(ysq[:], yp[:], ACT.Square, accum_out=dsD[:, col:col + 1])
        for ic in range(IC):
            nc.vector.tensor_reduce(dsD[:, D_RSUM + ic:D_RSUM + ic + 1],
                                    dsD[:, D_RS + ic * NB:D_RS + (ic + 1) * NB], axis=AX.X, op=OP.add)
        rs16 = dsb.tile([128, IC], BF16, tag="rs16")
        nc.scalar.copy(rs16[:], dsD[:, D_RSUM:D_RSUM + IC])
        for cc in range(CC):
            mup = dps.tile([128, 1], F32, tag="yp", bufs=2)
            for ic in range(IC):
                nc.tensor.matmul(mup[:], outwT[:, ic * C + cc * 128: ic * C + (cc + 1) * 128],
                                 rs16[:, ic:ic + 1], start=(ic == 0), stop=(ic == IC - 1))
            nc.vector.tensor_copy(dsD[:, D_ST + 2 * cc:D_ST + 2 * cc + 1], mup[:])
            nc.vector.tensor_reduce(dsD[:, D_ST + 2 * cc + 1:D_ST + 2 * cc + 2],
                                    dsD[:, D_SQ + cc * 2 * NB:D_SQ + (cc + 1) * 2 * NB],
                                    axis=AX.X, op=OP.add)

    if stop_after == "D":
        _finish_early()
        return

    # ---------------- collective ----------------
    with tc.tile_pool(name="cdram", bufs=1, space="DRAM") as cdram:
        arin = cdram.tile([128, 2 * CC], F32)
        arout = cdram.tile([128, 2 * CC], F32)
        nc.sync.dma_start(arin[:], dsD[:, D_ST:D_ST + 2 * CC])
        if use_collective:
            nc.gpsimd.collective_compute(
                "AllReduce", OP.add,
                ins=[arin.opt()], outs=[arout.opt()],
                replica_groups=[list(range(n_cores))],
            )
            nc.sync.dma_start(dsD[:, D_SBN:D_SBN + 2 * CC], arout[:])
        else:
            nc.sync.dma_start(dsD[:, D_SBN:D_SBN + 2 * CC], arin[:])

    # ---------------- BN finalize ----------------
    mom = dsD[:, D_MOM:D_MOM + 2 * CC]
    nc.scalar.mul(mom, dsD[:, D_SBN:D_SBN + 2 * CC], 1.0 / Ntot)
    muv = mom.rearrange("p (c two) -> p c two", two=2)[:, :, 0]
    msq = mom.rearrange("p (c two) -> p c two", two=2)[:, :, 1]
    nc.vector.tensor_mul(dsD[:, D_MUSQ:D_MUSQ + CC], muv, muv)
    nc.vector.tensor_sub(dsD[:, D_VAR:D_VAR + CC], msq, dsD[:, D_MUSQ:D_MUSQ + CC])
    nc.scalar.activation(dsD[:, D_SD:D_SD + CC], dsD[:, D_VAR:D_VAR + CC], ACT.Sqrt,
                         bias=wL[:, L_EPS:L_EPS + 1])
    nc.vector.reciprocal(dsD[:, D_RSTD:D_RSTD + CC], dsD[:, D_SD:D_SD + CC])
    scol = dsD[:, D_SCOL:D_SCOL + CC]
    bcol = dsD[:, D_BCOL:D_BCOL + CC]
    nc.vector.tensor_mul(scol, wL[:, L_GAMMA:L_GAMMA + CC], dsD[:, D_RSTD:D_RSTD + CC])
    nc.vector.tensor_scalar_mul(dsD[:, D_NSC:D_NSC + CC], scol, -1.0)
    for cc in range(CC):
        nc.vector.scalar_tensor_tensor(bcol[:, cc:cc + 1], muv[:, cc:cc + 1],
                                       dsD[:, D_NSC + cc:D_NSC + cc + 1],
                                       wL[:, L_BETA + cc:L_BETA + cc + 1], OP.mult, OP.add)

    if stop_after == "coll":
        _finish_early()
        return

    # ---------------- phase F ----------------
    with tc.tile_pool(name="phF_sb", bufs=1) as fsb, \
         tc.tile_pool(name="phF_ps", bufs=1, space="PSUM") as fps:
        for bi in range(BH):
            stage = [fsb.tile([128, RH * W], F32, tag=f"stage{cc}", name=f"stage{cc}")
                     for cc in range(CC)]
            for bj in range(BW):
                n = bi * BW + bj
                for cc in range(CC):
                    for nh in range(2):
                        yp = fps.tile([128, 512], F32, tag="yp2", bufs=2)
                        for ic in range(IC):
                            nc.tensor.matmul(yp[:], outwT[:, ic * C + cc * 128: ic * C + (cc + 1) * 128],
                                             atile[n][:, ic * P + nh * 512: ic * P + (nh + 1) * 512],
                                             start=(ic == 0), stop=(ic == IC - 1))
                        u = fsb.tile([128, 512], F32, tag="u_f", bufs=2)
                        nc.scalar.activation(u[:], yp[:], ACT.Identity,
                                             bias=bcol[:, cc:cc + 1], scale=scol[:, cc:cc + 1])
                        m = fsb.tile([128, 512], F32, tag="m_f", bufs=2)
                        nc.gpsimd.tensor_scalar_min(m[:], u[:], 0.0)
                        v = fsb.tile([128, 512], F32, tag="v_f", bufs=2)
                        nc.vector.scalar_tensor_tensor(v[:], m[:], wL[:, L_OAM1 + cc:L_OAM1 + cc + 1],
                                                       u[:], OP.mult, OP.add)
                        dst = stage[cc][:].rearrange("p (h w) -> p h w", w=W)[
                            :, 16 * nh:16 * (nh + 1), RW * bj:RW * (bj + 1)]
                        xres = x_sb[cc][:, n * P + nh * 512: n * P + (nh + 1) * 512]
                        nc.vector.tensor_add(dst, v[:], xres)
            for cc in range(CC):
                nc.sync.dma_start(yv[cc * 128:(cc + 1) * 128, RH * bi * W:RH * (bi + 1) * W], stage[cc][:])
    xpool.release()
    attw.release()
    dpool.release()
    wpoolL.release()


# ======================================================================
# Entry point: kernel(**inputs) -> np.ndarray [8, 512, 64, 128]
# ======================================================================
import concourse.bacc as bacc
import concourse.tile as tile
from concourse.bass_utils import run_bass_kernel_spmd

N_CORES = 8
_cached = {}


def _build_program(n_cores=N_CORES):
    if "nc" in _cached:
        return _cached["nc"]
    nc = bacc.Bacc("TRN2", target_bir_lowering=False, debug=False, num_devices=n_cores)
    ins = {"x": nc.dram_tensor("x", [C, HWp], BF16, kind="ExternalInput").ap()}
    for nm, shape, dt in WEIGHT_SPECS:
        ins[nm] = nc.dram_tensor(nm, shape, dt, kind="ExternalInput").ap()
    outs = {"y": nc.dram_tensor("y", [C, H, W], F32, kind="ExternalOutput").ap()}
    with tile.TileContext(nc) as tc:
        build_caam(tc, outs, ins, n_cores)
    nc.compile()
    _cached["nc"] = nc
    return nc


def pack_x(x):
    """[B, C, H, W] fp32 -> [B, C, HWp] bf16, bin-blocked (n*1024 + ph*32 + pw)."""
    xb = np.asarray(x, np.float32).reshape(B, C, BH, RH, BW, RW)
    xb = xb.transpose(0, 1, 2, 4, 3, 5).reshape(B, C, HWp)
    return np.ascontiguousarray(xb).astype(NPBF16)


def make_in_maps(inputs):
    xp = pack_x(inputs["x"])
    prep = host_prep(inputs)
    in_maps = []
    for c in range(N_CORES):
        d = {"x": np.ascontiguousarray(xp[c])}
        for nm, _, _ in WEIGHT_SPECS:
            d[nm] = prep[nm]
        in_maps.append(d)
    return in_maps


def kernel(**inputs):
    nc = _build_program()
    in_maps = make_in_maps(inputs)
    res = run_bass_kernel_spmd(nc, in_maps, core_ids=list(range(N_CORES)))
    return np.stack([res.results[c]["y"] for c in range(N_CORES)]).astype(np.float32)


# revision 31
# speedup vs baseline: 3.3634x; 3.3634x over previous
"""CAAM kernel for Trainium2: builder + host-side prep (bf16 matmul pipeline).

Per-core: one batch element. Key layouts:
  x resident as 4 SBUF tiles [128, 8192] bf16, BIN-BLOCKED on host
  (free index = n*1024 + ph*32 + pw), loaded with 4 contiguous DMAs.
  camE [32, 8192] bf16 (rows 19:32 zeroed): cam -> exp in place.
  Phase B uses DMA-transpose (XBAR, 16-bit) to produce pixel-partitioned
  ET [128, 8, 32] and xT [128, 8, 512] tiles — no PE transposes/copies.
  attnT is cached in SBUF (atile[n], bf16) through phase F, which therefore
  only redoes the out-projection matmul after BN stats are known.
  BN batch stats (sum, sumsq per channel) are allreduced across cores.
"""

import numpy as np
import ml_dtypes
import concourse.bass as bass
import concourse.mybir as mybir

F32 = mybir.dt.float32
BF16 = mybir.dt.bfloat16
NPBF16 = ml_dtypes.bfloat16
AX = mybir.AxisListType
OP = mybir.AluOpType
ACT = mybir.ActivationFunctionType

B, C, H, W = 8, 512, 64, 128
K, BH, BW = 19, 2, 4
NB = BH * BW          # 8
CI = C // 2           # 256
HWp = H * W           # 8192
RH, RW = H // BH, W // BW   # 32, 32
P = RH * RW           # 1024
CC = C // 128         # 4
IC = CI // 128        # 2
KN = K * NB           # 152
EPS = 1e-5

# -------- wpackE column map (fp32 consts) --------
E_IDN = 0        # 128 cols            identity (phase C transposes)
E_W1NK0 = 128    # 152 cols, rows 0:128  conv1 lhsT chunk0
E_W1NK1 = 280    # 152 cols, rows 0:24   conv1 lhsT chunk1
E_FNK0 = 432     # 19 cols, rows 0:128   fuse lhsT chunk0
E_FNK1 = 451     # 19 cols, rows 0:24    fuse lhsT chunk1
E_GANK = 470     # 2 cols: gcn_a-1 per stack row (chunk0, chunk1)
E_CAMB = 472     # 1 col, rows 0:19
E_FB = 473       # 1 col, rows 0:19      fuse_b
E_RAM1 = 474     # 1 col, rows 0:19      relu_a - 1
E_KB = 475       # 2 cols                k_b chunks
E_VB = 477       # 256 cols, row 0       v_b
E_ONE119 = 733   # 19 cols, row 0        ones
NEf = 752

# -------- wpackB column map (bf16 consts) --------
B_WCAM = 0       # 76 cols (4 chunks x 19)  conv_cam_w^T
B_ONEK = 76      # 1 col, rows 0:19  ones
B_ONE119 = 77    # 19 cols, row 0    ones
NBf = 96

# -------- wpackL column map (late fp32 consts, [128, 35]) --------
L_QB = 0         # 2 cols
L_GAMMA = 2      # 4
L_BETA = 6       # 4
L_OAM1 = 10      # 4  out_a - 1
L_EPS = 14       # 1
L_ONE1 = 16      # 1, row 0: 1.0 (outer-product transposes)
L_ONEC = 17      # 1, all rows: 1.0 (partition-sum matmuls)
NL = 35

# -------- dsmallA ([128, 40]): phase A stats --------
A_CSUM = 0       # 8 cols, rows 0:19
A_ESUM = 8
A_CLS = 16
A_REC = 24
A_SCALE = 32     # 8 cols: cls * rec
NA = 40

# -------- dsmallDE ([128, 646]) --------
D_RS = 0         # 16: attnT row sums (ic, bin)
D_SQ = 16        # 64: y^2 sums (cc, bin, nh)
D_ST = 80        # 8: packed allreduce input (sum, sumsq per cc)
D_SBN = 88       # 8: allreduce output
D_SCOL = 96      # 4
D_BCOL = 100     # 4
D_RSUM = 104     # 2
D_MOM = 106      # 8
D_VAR = 114      # 4
D_MUSQ = 118     # 4
D_SD = 122       # 4
D_RSTD = 126     # 4
D_NSC = 130      # 4
ND = 646

# -------- scrC column map (phase-C scratch inside bigE slot) --------
S_VA = 0         # 512   prelu'd t, chunk0
S_VB = 512       # 512, rows 0:24  chunk1
S_UG = 1024      # 512
S_MG = 1536      # 512
S_TT = 2048      # 608 = 4 x 152
S_L2A = 2656     # 512
S_L2B = 3168     # 512, rows 0:24
S_GL = 3680      # 512, rows 0:19  glob (prelu'd)
S_UG2 = 4192     # 512
S_MG2 = 4704     # 512
S_GT = 5216      # 76 = 4 x 19
S_L2T = 5292     # 608
NS = 5900

# attw pack (bf16): keyT 0:304, val 304:560
AT_KEYT = 0
AT_VAL = 304
NAT = 560


def host_prep(wts: dict) -> dict:
    w1 = np.asarray(wts["gcn_w1"], np.float32)
    ga = np.asarray(wts["gcn_a"], np.float32)
    fw = np.asarray(wts["fuse_w"], np.float32).reshape(-1)
    fb = float(np.asarray(wts["fuse_b"], np.float32).reshape(-1)[0])
    ra = float(np.asarray(wts["relu_a"], np.float32).reshape(-1)[0])

    wE = np.zeros((128, NEf), np.float32)
    # conv1 lhsT: W[(m*19+kp), (n*19+k)] = w1[n, m] * (kp == k)
    W1NK = np.zeros((KN, KN), np.float32)
    FNK = np.zeros((KN, K), np.float32)
    ga_nk = np.zeros(KN, np.float32)
    for n in range(NB):
        for k in range(K):
            for m in range(NB):
                W1NK[m*K + k, n*K + k] = w1[n, m]
            FNK[n*K + k, k] = fw[n]
            ga_nk[n*K + k] = ga[n] - 1.0
    wE[:, E_IDN:E_IDN + 128] = np.eye(128, dtype=np.float32)
    wE[:, E_W1NK0:E_W1NK0 + KN] = W1NK[0:128]
    wE[0:24, E_W1NK1:E_W1NK1 + KN] = W1NK[128:KN]
    wE[:, E_FNK0:E_FNK0 + K] = FNK[0:128]
    wE[0:24, E_FNK1:E_FNK1 + K] = FNK[128:KN]
    wE[:, E_GANK] = ga_nk[0:128]
    wE[0:24, E_GANK + 1] = ga_nk[128:KN]
    wE[0:K, E_CAMB] = np.asarray(wts["conv_cam_b"], np.float32)
    wE[0:K, E_FB] = fb
    wE[0:K, E_RAM1] = ra - 1.0
    wE[:, E_KB:E_KB + 2] = np.asarray(wts["k_b"], np.float32).reshape(IC, 128).T
    wE[0, E_VB:E_VB + CI] = np.asarray(wts["v_b"], np.float32)
    wE[0, E_ONE119:E_ONE119 + K] = 1.0

    wB = np.zeros((128, NBf), np.float32)
    wcamT = np.asarray(wts["conv_cam_w"], np.float32).T    # [512, 19]
    for cc in range(CC):
        wB[:, B_WCAM + cc*K:B_WCAM + (cc+1)*K] = wcamT[cc*128:(cc+1)*128]
    wB[0:K, B_ONEK] = 1.0
    wB[0, B_ONE119:B_ONE119 + K] = 1.0

    wL = np.zeros((128, NL), np.float32)
    wL[:, L_QB:L_QB + 2] = np.asarray(wts["q_b"], np.float32).reshape(IC, 128).T
    wL[:, L_GAMMA:L_GAMMA + 4] = np.asarray(wts["bn_gamma"], np.float32).reshape(CC, 128).T
    wL[:, L_BETA:L_BETA + 4] = np.asarray(wts["bn_beta"], np.float32).reshape(CC, 128).T
    wL[:, L_OAM1:L_OAM1 + 4] = (np.asarray(wts["out_a"], np.float32) - 1.0).reshape(CC, 128).T
    wL[:, L_EPS] = EPS
    wL[0, L_ONE1] = 1.0
    wL[:, L_ONEC] = 1.0

    return {
        "wpackE": wE,
        "wpackB": wB.astype(NPBF16),
        "wpackL": wL,
        "w2T": np.ascontiguousarray(np.asarray(wts["gcn_w2"], np.float32).T),
        "kwT": np.ascontiguousarray(np.asarray(wts["k_w"], np.float32).T),
        "vwT": np.ascontiguousarray(np.asarray(wts["v_w"], np.float32).T),
        "qwT": np.ascontiguousarray(np.asarray(wts["q_w"], np.float32).T).astype(NPBF16),
        "outwT": np.ascontiguousarray(np.asarray(wts["out_w"], np.float32).T).astype(NPBF16),
    }


WEIGHT_SPECS = [
    ("wpackE", [128, NEf], F32), ("wpackB", [128, NBf], BF16),
    ("wpackL", [128, NL], F32),
    ("w2T", [C, C], F32), ("kwT", [C, CI], F32), ("vwT", [C, CI], F32),
    ("qwT", [C, CI], BF16), ("outwT", [CI, C], BF16),
]


def _load_chunked(nc, pool, ap, r, cdim, name, dt=F32):
    """DRAM [r, cdim] (r = n*128) -> SBUF [128, n*cdim], column-grouped."""
    nchunk = r // 128
    t = pool.tile([128, nchunk * cdim], dt, name=name)
    src = ap.rearrange("(n p) c -> p n c", p=128)
    nc.sync.dma_start(t[:].rearrange("p (n c) -> p n c", n=nchunk), src)
    return t


def build_caam(tc, outs, ins, n_cores, use_collective=True, stop_after=None):
    nc = tc.nc
    x_d = ins["x"]          # [C, HWp] bf16, bin-blocked on host
    y_d = outs["y"]
    Ntot = float(n_cores * HWp)
    yv = y_d.rearrange("c h w -> c (h w)")

    # ---------------- pool stack (LIFO) ----------------
    wpoolL = tc.alloc_tile_pool(name="wtsL", bufs=1)
    dpool = tc.alloc_tile_pool(name="phD", bufs=1)
    attw = tc.alloc_tile_pool(name="attw", bufs=1)
    xpool = tc.alloc_tile_pool(name="x_res", bufs=1)
    gpool = tc.alloc_tile_pool(name="gcn", bufs=1)
    wpoolE = tc.alloc_tile_pool(name="wtsE", bufs=1)

    live = [wpoolL, dpool, attw, xpool, gpool, wpoolE]

    wE = wpoolE.tile([128, NEf], F32, name="wpackE")
    nc.sync.dma_start(wE[:], ins["wpackE"])
    wB = wpoolL.tile([128, NBf], BF16, name="wpackB")
    nc.sync.dma_start(wB[:], ins["wpackB"])
    kwT = _load_chunked(nc, wpoolE, ins["kwT"], C, CI, "kwT")
    vwT = _load_chunked(nc, wpoolE, ins["vwT"], C, CI, "vwT")
    wL = wpoolL.tile([128, NL], F32, name="wpackL")
    nc.sync.dma_start(wL[:], ins["wpackL"])
    qwT = _load_chunked(nc, wpoolL, ins["qwT"], C, CI, "qwT", dt=BF16)
    outwT = _load_chunked(nc, wpoolL, ins["outwT"], CI, C, "outwT", dt=BF16)

    idn = wE[:, E_IDN:E_IDN + 128]

    dsA = dpool.tile([128, NA], F32, name="dsmallA")
    dsD = dpool.tile([128, ND], F32, name="dsmallDE")

    def _finish_early():
        nc.vector.memset(dsD[:, 0:512], 0.0)
        nc.sync.dma_start(yv[0:128, 0:512], dsD[:, 0:512])
        for p in reversed(live):
            p.release()

    # ---------------- phase A ----------------
    x_sb = []
    for cc in range(CC):
        t = xpool.tile([128, HWp], BF16, name=f"x_{cc}")
        nc.sync.dma_start(t[:], x_d[cc * 128:(cc + 1) * 128, :])
        x_sb.append(t)

    if stop_after == "load":
        _finish_early()
        return

    camE = dpool.tile([32, HWp], BF16, tag="bigE", name="camE")
    nc.vector.memset(camE[:, :], 0.0)
    with tc.tile_pool(name="phA_ps", bufs=1, space="PSUM") as aps:
        for n in range(NB):
            cp = aps.tile([K, P], F32, tag="camps", bufs=2)
            for nh in range(2):
                for cc in range(CC):
                    nc.tensor.matmul(cp[:, nh * 512:(nh + 1) * 512],
                                     wB[:, B_WCAM + cc*K:B_WCAM + (cc+1)*K],
                                     x_sb[cc][:, n * P + nh * 512:n * P + (nh + 1) * 512],
                                     start=(cc == 0), stop=(cc == CC - 1))
            sl = camE[0:K, n * P:(n + 1) * P]
            nc.scalar.activation(sl, cp[:], ACT.Identity, bias=wE[0:K, E_CAMB:E_CAMB + 1])
            nc.vector.tensor_reduce(dsA[0:K, A_CSUM + n:A_CSUM + n + 1], sl, axis=AX.X, op=OP.add)
            nc.scalar.activation(sl, sl, ACT.Exp)
            nc.vector.tensor_reduce(dsA[0:K, A_ESUM + n:A_ESUM + n + 1], sl, axis=AX.X, op=OP.add)
    nc.scalar.activation(dsA[0:K, A_CLS:A_CLS + NB], dsA[0:K, A_CSUM:A_CSUM + NB],
                         ACT.Sigmoid, scale=1.0 / P)
    nc.vector.reciprocal(dsA[0:K, A_REC:A_REC + NB], dsA[0:K, A_ESUM:A_ESUM + NB])
    nc.vector.tensor_mul(dsA[0:K, A_SCALE:A_SCALE + NB],
                         dsA[0:K, A_CLS:A_CLS + NB], dsA[0:K, A_REC:A_REC + NB])

    if stop_after == "A":
        _finish_early()
        return

    # ---------------- phase B: per-bin local ----------------
    # stack rows: p = n*19 + k; chunk0 rows 0:128 cols 0:512, chunk1 rows 0:24 cols 512:1024
    stack = gpool.tile([128, 2 * C], F32, name="stack")
    stackA = stack[:, 0:C]
    stackB = stack[0:24, C:2 * C]
    with tc.tile_pool(name="phB_sb", bufs=1) as bsb, \
         tc.tile_pool(name="phB_ps", bufs=1, space="PSUM") as bps:
        for n in range(NB):
            # DMA-transpose (XBAR): ET[p, pc, k] = camE[k, pc*128+p]; xT[p, pc, c] = x[c, pc*128+p]
            ET = bsb.tile([128, NB, 32], BF16, tag="ET", bufs=2)
            nc.sync.dma_start(ET[:], camE[:, n * P:(n + 1) * P], transpose=True)
            xT = bsb.tile([128, NB, C], BF16, tag="xT", bufs=2)
            for cc in range(CC):
                nc.sync.dma_start(xT[:, :, cc * 128:(cc + 1) * 128],
                                  x_sb[cc][:, n * P:(n + 1) * P], transpose=True)
            locp = bps.tile([K, C], F32, tag="locp", bufs=2)
            for pc in range(8):
                nc.tensor.matmul(locp[:], ET[:, pc, 0:K], xT[:, pc, :],
                                 start=(pc == 0), stop=(pc == 7))
            locS = bsb.tile([K, C], F32, tag="locS", bufs=2)
            nc.vector.tensor_single_scalar(locS[:], locp[:],
                                           dsA[0:K, A_SCALE + n:A_SCALE + n + 1], OP.mult)
            # stack rows n*19 .. n*19+19 (may straddle the chunk boundary at p=128)
            p0 = n * K
            p1 = p0 + K
            if p1 <= 128:
                nc.sync.dma_start(stackA[p0:p1, :], locS[:, :])
            elif p0 >= 128:
                nc.sync.dma_start(stackB[p0 - 128:p1 - 128, :], locS[:, :])
            else:
                nc.sync.dma_start(stackA[p0:128, :], locS[0:128 - p0, :])
                nc.sync.dma_start(stackB[0:p1 - 128, :], locS[128 - p0:K, :])

    if stop_after == "B":
        _finish_early()
        return

    # ---------------- phase C: GCN + fuse + key/val (fp32) ----------------
    atp = attw.tile([128, NAT], BF16, name="attpack")
    keyT = atp[:, AT_KEYT:AT_KEYT + IC * KN]
    val = atp[0:K, AT_VAL:AT_VAL + CI]
    with tc.tile_pool(name="phC_sb", bufs=1) as csb, \
         tc.tile_pool(name="phC_ps", bufs=1, space="PSUM") as cps:
        scrC = csb.tile([128, NS], F32, name="scrC")
        vA = scrC[:, S_VA:S_VA + C]
        vB = scrC[0:24, S_VB:S_VB + C]
        w2T = _load_chunked(nc, csb, ins["w2T"], C, C, "w2T")
        # conv1: t = W1NK.T @ stack  (contraction over 152 stack rows, 2 chunks)
        tpA = cps.tile([128, C], F32, tag="big")
        nc.tensor.matmul(tpA[:], wE[:, E_W1NK0:E_W1NK0 + 128], stackA, start=True, stop=False)
        nc.tensor.matmul(tpA[:], wE[0:24, E_W1NK1:E_W1NK1 + 128], stackB, start=False, stop=True)
        tpB = cps.tile([24, C], F32, tag="smallB")
        nc.tensor.matmul(tpB[:], wE[:, E_W1NK0 + 128:E_W1NK0 + KN], stackA, start=True, stop=False)
        nc.tensor.matmul(tpB[:], wE[0:24, E_W1NK1 + 128:E_W1NK1 + KN], stackB, start=False, stop=True)
        # prelu(t + stack) with per-row alpha = gcn_a[n] (E_GANK cols)
        for (tp, st, vv, gchunk, rows) in ((tpA, stackA, vA, 0, 128),
                                           (tpB, stackB, vB, 1, 24)):
            u_ = scrC[0:rows, S_UG:S_UG + C]
            nc.vector.tensor_add(u_, tp[:], st)
            m_ = scrC[0:rows, S_MG:S_MG + C]
            nc.vector.tensor_scalar_min(m_, u_, 0.0)
            nc.vector.scalar_tensor_tensor(vv, m_, wE[0:rows, E_GANK + gchunk:E_GANK + gchunk + 1],
                                           u_, OP.mult, OP.add)
        # transpose t -> tT [c, (n,k)]
        for cc in range(CC):
            tt = scrC[:, S_TT + cc * KN:S_TT + (cc + 1) * KN]
            pA = cps.tile([128, 128], F32, tag="trA")
            nc.tensor.transpose(pA[:], vA[:, cc * 128:(cc + 1) * 128], idn)
            nc.scalar.copy(tt[:, 0:128], pA[:])
            pB = cps.tile([128, 24], F32, tag="trB")
            nc.tensor.transpose(pB[:], vB[:, cc * 128:(cc + 1) * 128], idn[0:24, 0:24])
            nc.scalar.copy(tt[:, 128:152], pB[:])
        # w2: local2 = t @ w2T (stack layout out)
        l2A = scrC[:, S_L2A:S_L2A + C]
        l2B = scrC[0:24, S_L2B:S_L2B + C]
        pl2A = cps.tile([128, C], F32, tag="big")
        for cc in range(CC):
            nc.tensor.matmul(pl2A[:], scrC[:, S_TT + cc * KN:S_TT + cc * KN + 128],
                             w2T[:, cc * C:(cc + 1) * C], start=(cc == 0), stop=(cc == CC - 1))
        nc.scalar.copy(l2A, pl2A[:])
        pl2B = cps.tile([24, C], F32, tag="smallB")
        for cc in range(CC):
            nc.tensor.matmul(pl2B[:], scrC[:, S_TT + cc * KN + 128:S_TT + cc * KN + 152],
                             w2T[:, cc * C:(cc + 1) * C], start=(cc == 0), stop=(cc == CC - 1))
        nc.scalar.copy(l2B, pl2B[:])
        # fuse -> glob [19, 512] (one psum tile), then prelu
        gp = cps.tile([K, C], F32, tag="gAB")
        nc.tensor.matmul(gp[:], wE[:, E_FNK0:E_FNK0 + K], l2A, start=True, stop=False)
        nc.tensor.matmul(gp[:], wE[0:24, E_FNK1:E_FNK1 + K], l2B, start=False, stop=True)
        glob = scrC[0:K, S_GL:S_GL + C]
        u_ = scrC[0:K, S_UG2:S_UG2 + C]
        nc.vector.tensor_scalar_add(u_, gp[:], wE[0:K, E_FB:E_FB + 1])
        m_ = scrC[0:K, S_MG2:S_MG2 + C]
        nc.vector.tensor_scalar_min(m_, u_, 0.0)
        nc.vector.scalar_tensor_tensor(glob, m_, wE[0:K, E_RAM1:E_RAM1 + 1], u_, OP.mult, OP.add)
        # globT + val (+ v_b via ones-row matmul); val cast to bf16
        valp = cps.tile([K, CI], F32, tag="valp")
        for cc in range(CC):
            gt = scrC[:, S_GT + cc * K:S_GT + (cc + 1) * K]
            pA = cps.tile([128, K], F32, tag="trB")
            nc.tensor.transpose(pA[:], glob[:, cc * 128:(cc + 1) * 128], idn[0:K, 0:K])
            nc.scalar.copy(gt[:, :], pA[:])
            nc.tensor.matmul(valp[:], gt[:], vwT[:, cc * CI:(cc + 1) * CI],
                             start=(cc == 0), stop=False)
        nc.tensor.matmul(valp[:], wE[0:1, E_ONE119:E_ONE119 + K], wE[0:1, E_VB:E_VB + CI],
                         start=False, stop=True)
        nc.scalar.copy(val, valp[:])
        # local2T + keyT (+ k_b per-partition bias); keyT cast to bf16
        for cc in range(CC):
            lt = scrC[:, S_L2T + cc * KN:S_L2T + (cc + 1) * KN]
            pA = cps.tile([128, 128], F32, tag="trA")
            nc.tensor.transpose(pA[:], l2A[:, cc * 128:(cc + 1) * 128], idn)
            nc.scalar.copy(lt[:, 0:128], pA[:])
            pB = cps.tile([128, 24], F32, tag="trB")
            nc.tensor.transpose(pB[:], l2B[:, cc * 128:(cc + 1) * 128], idn[0:24, 0:24])
            nc.scalar.copy(lt[:, 128:152], pB[:])
        for ic in range(IC):
            kp = cps.tile([128, KN], F32, tag="keyp")
            for cc in range(CC):
                nc.tensor.matmul(kp[:], kwT[:, cc * CI + ic * 128: cc * CI + (ic + 1) * 128],
                                 scrC[:, S_L2T + cc * KN:S_L2T + (cc + 1) * KN],
                                 start=(cc == 0), stop=(cc == CC - 1))
            nc.scalar.activation(keyT[:, ic * KN:(ic + 1) * KN], kp[:], ACT.Identity,
                                 bias=wE[:, E_KB + ic:E_KB + ic + 1])
    wpoolE.release()
    gpool.release()
    live.remove(wpoolE)
    live.remove(gpool)

    if stop_after == "C":
        _finish_early()
        return

    # ---------------- phase D+E: attention, y (once, into ybuf), Gram stats ----
    # sumsq_c = diag(W G W^T) with G = V^T (sum_p e e^T) V: one 19x19 Gram of
    # the normalized attention weights replaces a second full y pass.
    ybuf = [dpool.tile([128, HWp], BF16, tag="bigE", name="ybuf0")]
    for cc in range(1, CC):
        yb = dpool.tile([128, HWp], BF16, name=f"ybuf{cc}")
        ybuf.append(yb)
    with tc.tile_pool(name="phD_sb", bufs=1) as dsb, \
         tc.tile_pool(name="phD_ps", bufs=1, space="PSUM") as dps:
        Ge = dps.tile([K, K], F32, tag="Ge")
        for n in range(NB):
            qT = dsb.tile([128, IC * P], BF16, tag="qT", bufs=2)
            for ic in range(IC):
                for nh in range(2):
                    qp = dps.tile([128, 512], F32, tag="qp")
                    for cc in range(CC):
                        xsl = x_sb[cc][:, n * P + nh * 512: n * P + (nh + 1) * 512]
                        nc.tensor.matmul(qp[:], qwT[:, cc * CI + ic * 128: cc * CI + (ic + 1) * 128],
                                         xsl, start=(cc == 0), stop=(cc == CC - 1))
                    nc.scalar.activation(qT[:, ic * P + nh * 512: ic * P + (nh + 1) * 512], qp[:],
                                         ACT.Identity, bias=wL[:, L_QB + ic:L_QB + ic + 1])
            ebin = dsb.tile([32, P], BF16, tag="ebin", bufs=2)
            nc.vector.memset(ebin[:, :], 0.0)
            for nh in range(2):
                afp = dps.tile([K, 512], F32, tag="afp")
                for ic in range(IC):
                    ksel = keyT[:, ic * KN + n * K: ic * KN + (n + 1) * K]
                    nc.tensor.matmul(afp[:], ksel, qT[:, ic * P + nh * 512: ic * P + (nh + 1) * 512],
                                     start=(ic == 0), stop=(ic == IC - 1))
                nc.scalar.activation(ebin[0:K, nh * 512:(nh + 1) * 512], afp[:], ACT.Exp)
                sp = dps.tile([1, 512], F32, tag="nrm")
                nc.tensor.matmul(sp[:], wB[0:K, B_ONEK:B_ONEK + 1],
                                 ebin[0:K, nh * 512:(nh + 1) * 512], start=True, stop=True)
                rrow = dsb.tile([1, 512], BF16, tag="rrow", bufs=2)
                with nc.allow_low_precision(reason="softmax denom reciprocal; 2e-2 tol"):
                    nc.vector.reciprocal(rrow[:], sp[:])
                rbp = dps.tile([K, 512], F32, tag="nrm")
                nc.tensor.matmul(rbp[:], wB[0:1, B_ONE119:B_ONE119 + K], rrow[:], start=True, stop=True)
                nc.vector.tensor_mul(ebin[0:K, nh * 512:(nh + 1) * 512],
                                     ebin[0:K, nh * 512:(nh + 1) * 512], rbp[:])
            # Gram accumulation over all bins/pixels (zero rows 19:32 are inert)
            eT = dsb.tile([128, NB, 32], BF16, tag="eT", bufs=2)
            nc.sync.dma_start(eT[:], ebin[:, :], transpose=True)
            for pc in range(8):
                nc.tensor.matmul(Ge[:], eT[:, pc, 0:K], eT[:, pc, 0:K],
                                 start=(n == 0 and pc == 0), stop=(n == NB - 1 and pc == 7),
                                 skip_group_check=True)
            atile = dsb.tile([128, IC * P], BF16, tag="atile", bufs=2)
            for ic in range(IC):
                aop = dps.tile([128, P], F32, tag="aop")
                for nh in range(2):
                    nc.tensor.matmul(aop[:, nh * 512:(nh + 1) * 512], val[:, ic * 128:(ic + 1) * 128],
                                     ebin[0:K, nh * 512:(nh + 1) * 512], start=True, stop=True)
                nc.scalar.activation(atile[:, ic * P:(ic + 1) * P], aop[:], ACT.Copy,
                                     accum_out=dsD[:, D_RS + ic * NB + n: D_RS + ic * NB + n + 1])
            for cc in range(CC):
                for nh in range(2):
                    yp = dps.tile([128, 512], F32, tag="yp", bufs=2)
                    for ic in range(IC):
                        nc.tensor.matmul(yp[:], outwT[:, ic * C + cc * 128: ic * C + (cc + 1) * 128],
                                         atile[:, ic * P + nh * 512: ic * P + (nh + 1) * 512],
                                         start=(ic == 0), stop=(ic == IC - 1))
                    nc.scalar.activation(ybuf[cc][:, n * P + nh * 512: n * P + (nh + 1) * 512],
                                         yp[:], ACT.Copy)
        # mu path: attnT row sums -> W @ rowsum
        for ic in range(IC):
            nc.vector.tensor_reduce(dsD[:, D_RSUM + ic:D_RSUM + ic + 1],
                                    dsD[:, D_RS + ic * NB:D_RS + (ic + 1) * NB], axis=AX.X, op=OP.add)
        rs16 = dsb.tile([128, IC], BF16, tag="rs16")
        nc.scalar.copy(rs16[:], dsD[:, D_RSUM:D_RSUM + IC])
        for cc in range(CC):
            mup = dps.tile([128, 1], F32, tag="yp", bufs=2)
            for ic in range(IC):
                nc.tensor.matmul(mup[:], outwT[:, ic * C + cc * 128: ic * C + (cc + 1) * 128],
                                 rs16[:, ic:ic + 1], start=(ic == 0), stop=(ic == IC - 1))
            nc.vector.tensor_copy(dsD[:, D_ST + 2 * cc:D_ST + 2 * cc + 1], mup[:])
        # sumsq via Gram: G' = V^T Ge V  (2x[128,256]); MT = G' @ outwT;
        # sumsq = colsum(MT . outwT), then transposed into D_ST odd cols.
        GeS = dsb.tile([K, K], BF16, tag="GeS")
        nc.scalar.copy(GeS[:], Ge[:])
        Hp = dps.tile([K, CI], F32, tag="qp")
        nc.tensor.matmul(Hp[:], GeS[:], val[:, :], start=True, stop=True)
        Hs = dsb.tile([K, CI], BF16, tag="Hs")
        nc.scalar.copy(Hs[:], Hp[:])
        Gs = dsb.tile([128, IC * CI], BF16, tag="Gs")
        for i1c in range(IC):
            Gp = dps.tile([128, CI], F32, tag="qp")
            nc.tensor.matmul(Gp[:], val[:, i1c * 128:(i1c + 1) * 128], Hs[:], start=True, stop=True)
            nc.scalar.copy(Gs[:, i1c * CI:(i1c + 1) * CI], Gp[:])
        prod = dsb.tile([128, IC * C], F32, tag="prod")
        for jc in range(IC):
            MTp = dps.tile([128, C], F32, tag="aop")
            for i2c in range(IC):
                nc.tensor.matmul(MTp[:], Gs[:, i2c * CI + jc * 128: i2c * CI + (jc + 1) * 128],
                                 outwT[:, i2c * C:(i2c + 1) * C], start=(i2c == 0), stop=(i2c == IC - 1))
            nc.vector.tensor_mul(prod[:, jc * C:(jc + 1) * C], MTp[:], outwT[:, jc * C:(jc + 1) * C])
        sq_ps = dps.tile([1, C], F32, tag="nrm")
        for jc in range(IC):
            nc.tensor.matmul(sq_ps[:], wL[:, L_ONEC:L_ONEC + 1], prod[:, jc * C:(jc + 1) * C],
                             start=(jc == 0), stop=(jc == IC - 1))
        sqrow = dsb.tile([1, C], F32, tag="sqrow")
        nc.vector.tensor_copy(sqrow[:], sq_ps[:])
        for cc in range(CC):
            tp = dps.tile([128, 1], F32, tag="yp", bufs=2)
            nc.tensor.matmul(tp[:], sqrow[0:1, cc * 128:(cc + 1) * 128],
                             wL[0:1, L_ONE1:L_ONE1 + 1], start=True, stop=True)
            nc.vector.tensor_copy(dsD[:, D_ST + 2 * cc + 1:D_ST + 2 * cc + 2], tp[:])

    if stop_after == "D":
        _finish_early()
        return

    # ---------------- collective ----------------
    with tc.tile_pool(name="cdram", bufs=1, space="DRAM") as cdram:
        arin = cdram.tile([128, 2 * CC], F32)
        arout = cdram.tile([128, 2 * CC], F32)
        nc.sync.dma_start(arin[:], dsD[:, D_ST:D_ST + 2 * CC])
        if use_collective:
            nc.gpsimd.collective_compute(
                "AllReduce", OP.add,
                ins=[arin.opt()], outs=[arout.opt()],
                replica_groups=[list(range(n_cores))],
            )
            nc.sync.dma_start(dsD[:, D_SBN:D_SBN + 2 * CC], arout[:])
        else:
            nc.sync.dma_start(dsD[:, D_SBN:D_SBN + 2 * CC], arin[:])

    # ---------------- BN finalize ----------------
    mom = dsD[:, D_MOM:D_MOM + 2 * CC]
    nc.scalar.mul(mom, dsD[:, D_SBN:D_SBN + 2 * CC], 1.0 / Ntot)
    muv = mom.rearrange("p (c two) -> p c two", two=2)[:, :, 0]
    msq = mom.rearrange("p (c two) -> p c two", two=2)[:, :, 1]
    nc.vector.tensor_mul(dsD[:, D_MUSQ:D_MUSQ + CC], muv, muv)
    nc.vector.tensor_sub(dsD[:, D_VAR:D_VAR + CC], msq, dsD[:, D_MUSQ:D_MUSQ + CC])
    nc.scalar.activation(dsD[:, D_SD:D_SD + CC], dsD[:, D_VAR:D_VAR + CC], ACT.Sqrt,
                         bias=wL[:, L_EPS:L_EPS + 1])
    nc.vector.reciprocal(dsD[:, D_RSTD:D_RSTD + CC], dsD[:, D_SD:D_SD + CC])
    scol = dsD[:, D_SCOL:D_SCOL + CC]
    bcol = dsD[:, D_BCOL:D_BCOL + CC]
    nc.vector.tensor_mul(scol, wL[:, L_GAMMA:L_GAMMA + CC], dsD[:, D_RSTD:D_RSTD + CC])
    nc.vector.tensor_scalar_mul(dsD[:, D_NSC:D_NSC + CC], scol, -1.0)
    for cc in range(CC):
        nc.vector.scalar_tensor_tensor(bcol[:, cc:cc + 1], muv[:, cc:cc + 1],
                                       dsD[:, D_NSC + cc:D_NSC + cc + 1],
                                       wL[:, L_BETA + cc:L_BETA + cc + 1], OP.mult, OP.add)

    if stop_after == "coll":
        _finish_early()
        return

    # ---------------- phase F: BN affine + PReLU + residual (no matmuls) -----
    with tc.tile_pool(name="phF_sb", bufs=1) as fsb:
        for bi in range(BH):
            for cp in range(2):
                stage = [fsb.tile([128, RH * W], F32, tag=f"stage{q}", name=f"stage{q}")
                         for q in range(2)]
                for bj in range(BW):
                    n = bi * BW + bj
                    for q in range(2):
                        cc = cp * 2 + q
                        for nh in range(2):
                            ysl = ybuf[cc][:, n * P + nh * 512: n * P + (nh + 1) * 512]
                            u = fsb.tile([128, 512], F32, tag="u_f", bufs=2)
                            nc.scalar.activation(u[:], ysl, ACT.Identity,
                                                 bias=bcol[:, cc:cc + 1], scale=scol[:, cc:cc + 1])
                            m = fsb.tile([128, 512], F32, tag="m_f", bufs=2)
                            nc.gpsimd.tensor_scalar_min(m[:], u[:], 0.0)
                            v = fsb.tile([128, 512], F32, tag="v_f", bufs=2)
                            nc.vector.scalar_tensor_tensor(v[:], m[:], wL[:, L_OAM1 + cc:L_OAM1 + cc + 1],
                                                           u[:], OP.mult, OP.add)
                            dst = stage[q][:].rearrange("p (h w) -> p h w", w=W)[
                                :, 16 * nh:16 * (nh + 1), RW * bj:RW * (bj + 1)]
                            xres = x_sb[cc][:, n * P + nh * 512: n * P + (nh + 1) * 512]
                            nc.vector.tensor_add(dst, v[:], xres)
                for q in range(2):
                    cc = cp * 2 + q
                    nc.sync.dma_start(yv[cc * 128:(cc + 1) * 128, RH * bi * W:RH * (bi + 1) * W],
                                      stage[q][:])
    xpool.release()
    attw.release()
    dpool.release()
    wpoolL.release()


# ======================================================================
# Entry point: kernel(**inputs) -> np.ndarray [8, 512, 64, 128]
# ======================================================================
import concourse.bacc as bacc
import concourse.tile as tile
from concourse.bass_utils import run_bass_kernel_spmd

N_CORES = 8
_cached = {}


def _build_program(n_cores=N_CORES):
    if "nc" in _cached:
        return _cached["nc"]
    nc = bacc.Bacc("TRN2", target_bir_lowering=False, debug=False, num_devices=n_cores)
    ins = {"x": nc.dram_tensor("x", [C, HWp], BF16, kind="ExternalInput").ap()}
    for nm, shape, dt in WEIGHT_SPECS:
        ins[nm] = nc.dram_tensor(nm, shape, dt, kind="ExternalInput").ap()
    outs = {"y": nc.dram_tensor("y", [C, H, W], F32, kind="ExternalOutput").ap()}
    with tile.TileContext(nc) as tc:
        build_caam(tc, outs, ins, n_cores)
    nc.compile()
    _cached["nc"] = nc
    return nc


def pack_x(x):
    """[B, C, H, W] fp32 -> [B, C, HWp] bf16, bin-blocked (n*1024 + ph*32 + pw)."""
    xb = np.asarray(x, np.float32).reshape(B, C, BH, RH, BW, RW)
    xb = xb.transpose(0, 1, 2, 4, 3, 5).reshape(B, C, HWp)
    return np.ascontiguousarray(xb).astype(NPBF16)


def make_in_maps(inputs):
    xp = pack_x(inputs["x"])
    prep = host_prep(inputs)
    in_maps = []
    for c in range(N_CORES):
        d = {"x": np.ascontiguousarray(xp[c])}
        for nm, _, _ in WEIGHT_SPECS:
            d[nm] = prep[nm]
        in_maps.append(d)
    return in_maps


def kernel(**inputs):
    nc = _build_program()
    in_maps = make_in_maps(inputs)
    res = run_bass_kernel_spmd(nc, in_maps, core_ids=list(range(N_CORES)))
    return np.stack([res.results[c]["y"] for c in range(N_CORES)]).astype(np.float32)
